# revision 13
# baseline (speedup 1.0000x reference)
"""DeepWalk loss kernel for 8 Trainium2 NeuronCores.

Strategy: data-parallel over the 512 walks (64 walks per core). Each core
builds an on-chip bf16 "token table" of the gathered embedding rows
(node||ctx, 512B per token) straight from HBM with windowed dma_gather
(windows of 32768 rows so indices fit int16), then computes all pair dot
products with SBUF-source dma_gather (transposed layout) -> DVE multiply ->
PE sliding-window ones-matmul reduction into PSUM banks -> softplus
(Exp + Ln LUTs) with fused row-sum accumulation. Host sums the 8x[128]
partial sums and divides by the pair count.
"""

import os
import sys

import numpy as np
import ml_dtypes

sys.path.insert(0, "/opt/trn_rl_repo")

import concourse.bacc as bacc
import concourse.bass as bass
import concourse.mybir as mybir
import concourse.tile as tile
from concourse import library_config
from concourse.bass_utils import run_bass_kernel_spmd
from concourse._compat import with_exitstack

BF16 = ml_dtypes.bfloat16

# Problem constants (hardcoded per the harness contract).
EMB_DIM = 128
WALK_LEN = 40
WINDOW = 5
NEG_SIZE = 5
NUM_NODES = 1_000_000
BATCH = 512
N_CORES = 8

NB_CORE = BATCH // N_CORES            # 64 walks per core
NLOC = NB_CORE * WALK_LEN             # 2560 local walk positions
GTOK = BATCH * WALK_LEN               # 20480 global walk positions
P = 128
WIN_ROWS = 32768                      # int16-addressable window of the table

def _pair_indices():
    src, dst = [], []
    for i in range(WALK_LEN):
        for j in range(max(0, i - WINDOW), i):
            src.append(j); dst.append(i)
        for j in range(i + 1, min(WALK_LEN, i + 1 + WINDOW)):
            src.append(j); dst.append(i)
    return np.asarray(src, dtype=np.int64), np.asarray(dst, dtype=np.int64)

_SRC, _DST = _pair_indices()
NUM_PAIRS = _SRC.shape[0]             # 370
POS_CORE = NB_CORE * NUM_PAIRS        # 23680 positive pairs per core
SLICE = 512
NSLOT_STREAM = (POS_CORE + SLICE - 1) // SLICE   # 47 slices per score stream
NPAD = NSLOT_STREAM * SLICE           # 24064 padded pairs per stream
CHUNK = 2560                          # super-chunk (pairs); 2560 = 5*512 = 20*128
CHUNKS = [CHUNK] * (NPAD // CHUNK) + ([NPAD % CHUNK] if NPAD % CHUNK else [])
N_STREAMS = 6                         # pos, neg j=0..4
TOT_SLOTS = N_STREAMS * NSLOT_STREAM  # 282
IDX_COLS = NPAD // 16                 # 1504 idx columns per list
N_LISTS = 7                           # AB, C, D0..D4
BANK_ROWS = [128, 128, TOT_SLOTS - 256]   # 128/128/26


def _wrap16(a):
    """int16 list [N] -> [128, N/16] dma_gather idx layout (16-wrap, 8x replicated)."""
    a = a.astype(np.int16)
    t = a.reshape(-1, 16).T          # [16, N/16]
    return np.tile(t, (8, 1)).copy() # [128, N/16]


def _plan(fw, vocab):
    """Window build plan: group the 20480 walk rows by table window.

    Returns (padw, nwin, widx_lists, pos) where widx_lists[i] is the padded
    int array of in-window row offsets gathered by window call i, and pos[w]
    is the token id assigned to walk position w.
    """
    nwin = (vocab + WIN_ROWS - 1) // WIN_ROWS
    win = fw // WIN_ROWS
    counts = np.bincount(win, minlength=nwin)
    padw = max(896, int(-(-counts.max() // P) * P))
    pos = np.empty(GTOK, dtype=np.int32)
    widx_lists = []
    for i in range(nwin):
        ws = np.nonzero(win == i)[0]
        pos[ws] = padw * i + np.arange(len(ws), dtype=np.int32)
        lst = np.zeros(padw, dtype=np.int32)
        lst[:len(ws)] = fw[ws] - WIN_ROWS * i
        widx_lists.append(lst)
    return padw, nwin, widx_lists, pos


def _host_prepare(batch_walk, neg_idx_dst, node_embed, context_embed):
    """Sharding/index prep. Index arithmetic + dtype casts only."""
    fw = np.asarray(batch_walk).reshape(-1).astype(np.int32)       # [20480]
    neg = np.asarray(neg_idx_dst).astype(np.int32)                 # [947200]
    vocab = int(np.asarray(node_embed).shape[0])

    tok = np.empty((vocab, 2 * EMB_DIM), dtype=BF16)
    tok[:, :EMB_DIM] = np.asarray(node_embed).astype(BF16)
    tok[:, EMB_DIM:] = np.asarray(context_embed).astype(BF16)

    padw, nwin, widx_lists, pos = _plan(fw, vocab)
    ntokb = padw * nwin
    pad_x, pad_y, pad_z = ntokb, ntokb + 1, ntokb + 2
    assert ntokb + 3 <= 32768, "token ids must fit int16"

    ptok = np.zeros((3, 2 * EMB_DIM), dtype=BF16)
    ptok[0, 0] = 1.0; ptok[0, EMB_DIM] = 30.0   # X: node=[1,..], ctx=[30,..]
    ptok[1, 0] = 1.0                            # Y: node=[1,..]
    ptok[2, EMB_DIM] = -30.0                    # Z: ctx=[-30,..]

    widx = np.concatenate([_wrap16(a) for a in widx_lists], axis=1)

    bl = np.repeat(np.arange(NB_CORE, dtype=np.int32), NUM_PAIRS)
    qq = np.tile(np.arange(NUM_PAIRS, dtype=np.int32), NB_CORE)
    npad = NPAD - POS_CORE

    in_maps = []
    for k in range(N_CORES):
        wloc = k * NLOC  # this core's batches start at walk position k*2560
        ab_t = pos[wloc + bl * WALK_LEN + _DST[qq].astype(np.int32)]
        c_t = pos[wloc + bl * WALK_LEN + _SRC[qq].astype(np.int32)]
        ab = np.concatenate([ab_t, np.full(npad, pad_x, np.int32)])
        cc = np.concatenate([c_t, np.full(npad, pad_y, np.int32)])
        negk = neg[k * POS_CORE * NEG_SIZE:(k + 1) * POS_CORE * NEG_SIZE]
        negk = negk.reshape(POS_CORE, NEG_SIZE)
        lists = [ab, cc]
        for j in range(NEG_SIZE):
            dj = np.concatenate([pos[negk[:, j]], np.full(npad, pad_z, np.int32)])
            lists.append(dj)
        gidx = np.concatenate([_wrap16(a) for a in lists], axis=1)  # [128, 7*1504]
        in_maps.append({"tok": tok, "widx": widx, "gidx": gidx, "ptok": ptok})
    return in_maps, padw, nwin


@with_exitstack
def _body(ctx, tc, nc, tok_t, widx_t, gidx_t, ptok_t, out_t, vocab, padw, nwin):
    dt = mybir.dt
    ntokb = padw * nwin
    nranks = (ntokb + 3 + P - 1) // P
    wranks = padw // P
    wcols = padw // 16

    tabp = ctx.enter_context(tc.tile_pool(name="tab", bufs=1))
    cst = ctx.enter_context(tc.tile_pool(name="cst", bufs=1))
    abp = ctx.enter_context(tc.tile_pool(name="ab", bufs=2))
    cp = ctx.enter_context(tc.tile_pool(name="cpool", bufs=2))
    dp = ctx.enter_context(tc.tile_pool(name="dpool", bufs=3))
    psp = ctx.enter_context(tc.tile_pool(name="ps", bufs=3, space="PSUM"))
    scp = ctx.enter_context(tc.tile_pool(name="scr", bufs=2))

    table = tabp.tile([P, nranks, 2 * EMB_DIM], dt.bfloat16)
    widx = cst.tile([P, nwin * wcols], dt.int16)
    gidx = cst.tile([P, N_LISTS * IDX_COLS], dt.int16)
    w_one = cst.tile([P, 257], dt.bfloat16)
    w_neg = cst.tile([P, 257], dt.bfloat16)

    nc.sync.dma_start(widx[:], widx_t[:])
    nc.sync.dma_start(gidx[:], gidx_t[:])
    nc.vector.memset(w_one[:, :], 0.0)
    nc.vector.memset(w_one[:, 128:129], 1.0)
    nc.vector.memset(w_neg[:, :], 0.0)
    nc.vector.memset(w_neg[:, 128:129], -1.0)

    # Build the token table: window i gathers its rows (full 512B node||ctx
    # tokens) into ranks [i*wranks, (i+1)*wranks); pad tokens land in the
    # final rank (zeroed first so every table byte is initialized).
    nc.vector.memset(table[:, nranks - 1:nranks, :], 0.0)
    nc.sync.dma_start(table[0:3, nranks - 1:nranks, :], ptok_t[:])
    for i in range(nwin):
        lo = WIN_ROWS * i
        hi = min(vocab, lo + WIN_ROWS)
        for off in range(0, padw, CHUNK):
            n = min(CHUNK, padw - off)
            r0 = (padw * i + off) // P
            nc.gpsimd.dma_gather(
                table[:, r0:r0 + n // P, :],
                tok_t[lo:hi, :],
                widx[:, i * wcols + off // 16:i * wcols + (off + n) // 16],
                n, n, 2 * EMB_DIM,
                single_packet=False,
            )

    def gather(dst, cols, n, elem, boff):
        nc.gpsimd.dma_gather(
            dst[:], table[:], gidx[:, cols:cols + n // 16], n, n, elem,
            transpose=True,
            single_packet=False,
            sbuf_tokens_per_rank=128,
            sbuf_free_dim_per_rank=4 * EMB_DIM,
            sbuf_byte_offset=boff,
        )

    banks = []
    slot = 0

    def mm(rhs_ap, is_pos):
        nonlocal slot
        if slot % 128 == 0:
            bank = psp.tile([P, SLICE], dt.float32, space="PSUM", tag="bank")
            banks.append(bank)
        r = slot % 128
        w = w_neg if is_pos else w_one
        nc.tensor.matmul(
            out=banks[-1][:],
            lhsT=w[:, 128 - r:256 - r],
            rhs=rhs_ap,
            start=(r == 0),
            stop=(r == 127 or slot == TOT_SLOTS - 1),
        )
        slot += 1

    col0 = 0
    for ch in CHUNKS:
        ncols = ch // 16
        ab = abp.tile([P, 2, ch], dt.bfloat16, tag="ab")
        gather(ab, col0, ch, 2 * EMB_DIM, 0)
        c = cp.tile([P, 1, ch], dt.bfloat16, tag="c")
        gather(c, IDX_COLS + col0, ch, EMB_DIM, 0)
        nc.vector.tensor_mul(c[:, 0, :ch], c[:, 0, :ch], ab[:, 1, :ch])
        for si in range(ch // SLICE):
            mm(c[:, 0, si * SLICE:(si + 1) * SLICE], True)
        for j in range(NEG_SIZE):
            d = dp.tile([P, 1, ch], dt.bfloat16, tag="d")
            gather(d, (2 + j) * IDX_COLS + col0, ch, EMB_DIM, 2 * EMB_DIM)
            nc.vector.tensor_mul(d[:, 0, :ch], d[:, 0, :ch], ab[:, 0, :ch])
            for si in range(ch // SLICE):
                mm(d[:, 0, si * SLICE:(si + 1) * SLICE], False)
        col0 += ncols

    accvs = []
    for bi, bank in enumerate(banks):
        rows = BANK_ROWS[bi]
        e = scp.tile([P, SLICE], dt.float32, tag="e")
        sp = scp.tile([P, SLICE], dt.float32, tag="sp")
        av = cst.tile([P, 1], dt.float32, tag=f"av{bi}")
        nc.scalar.activation(e[:rows, :], bank[:rows, :],
                             mybir.ActivationFunctionType.Exp)
        nc.scalar.activation(sp[:rows, :], e[:rows, :],
                             mybir.ActivationFunctionType.Ln,
                             bias=1.0, accum_out=av[:rows, :])
        accvs.append(av)
    osb = cst.tile([P, 1], dt.float32, tag="osb")
    nc.vector.memset(osb[:], 0.0)
    for bi, av in enumerate(accvs):
        r = BANK_ROWS[bi]
        nc.vector.tensor_add(osb[:r, :], osb[:r, :], av[:r, :])
    nc.sync.dma_start(out_t[:], osb[:])


def _build_program(loop_k, vocab, padw, nwin):
    nc = bacc.Bacc("TRN2", target_bir_lowering=False, debug=False)
    tok_t = nc.dram_tensor("tok", [vocab, 2 * EMB_DIM], mybir.dt.bfloat16,
                           kind="ExternalInput")
    widx_t = nc.dram_tensor("widx", [P, nwin * padw // 16], mybir.dt.int16,
                            kind="ExternalInput")
    gidx_t = nc.dram_tensor("gidx", [P, N_LISTS * IDX_COLS], mybir.dt.int16,
                            kind="ExternalInput")
    ptok_t = nc.dram_tensor("ptok", [3, 2 * EMB_DIM], mybir.dt.bfloat16,
                            kind="ExternalInput")
    out_t = nc.dram_tensor("out", [P, 1], mybir.dt.float32, kind="ExternalOutput")
    with tile.TileContext(nc) as tc:
        nc.gpsimd.load_library(library_config.mlp)
        if loop_k is None:
            _body(tc, nc, tok_t, widx_t, gidx_t, ptok_t, out_t, vocab, padw, nwin)
        else:
            tc.For_i_unrolled(0, loop_k, 1,
                              lambda iv: _body(tc, nc, tok_t, widx_t, gidx_t,
                                               ptok_t, out_t, vocab, padw, nwin),
                              max_unroll=1)
    nc.compile()
    return nc


_CACHE = {}


def _get_program(loop_k, vocab, padw, nwin):
    key = (loop_k, vocab, padw, nwin)
    if key not in _CACHE:
        _CACHE[key] = _build_program(loop_k, vocab, padw, nwin)
    return _CACHE[key]


def run_cores(inputs, loop_k=None):
    """Run the SPMD kernel; returns list of per-core [128,1] partial sums."""
    in_maps, padw, nwin = _host_prepare(**inputs)
    vocab = int(np.asarray(inputs["node_embed"]).shape[0])
    nc = _get_program(loop_k, vocab, padw, nwin)
    res = run_bass_kernel_spmd(nc, in_maps, core_ids=list(range(N_CORES)))
    return [res.results[i]["out"] for i in range(N_CORES)]


def kernel(batch_walk, neg_idx_dst, node_embed, context_embed):
    outs = run_cores(dict(batch_walk=batch_walk, neg_idx_dst=neg_idx_dst,
                          node_embed=node_embed, context_embed=context_embed))
    total = float(sum(float(o.sum()) for o in outs))
    return np.float32(total / (BATCH * NUM_PAIRS))
